# revision 29
# baseline (speedup 1.0000x reference)
"""APPNP GNN (2-layer MLP + 5 rounds of weighted message passing + log_softmax)
on 8 Trainium2 NeuronCores.

Strategy (dst-partitioned message passing):
  - Nodes are sharded 8 ways by row range; core c owns dst rows
    [c*12500, (c+1)*12500).
  - MLP (x @ W1 -> relu -> @ W2) is computed data-parallel over the node
    shard, in bf16 with fp32 PSUM accumulation. The host passes x
    pre-transposed (feature-major) so no on-device transposes of x are
    needed; the MLP directly produces feature-major h which is PE-transposed
    back to node-major 128x128 tiles.
  - The propagated table H [100000, 128] lives in DRAM (bf16), replicated
    per core via AllGather after each hop.
  - Per hop, each core processes its incident edges (host-sorted by dst):
    a large indirect DMA gathers h[src] rows (256B each) for 128-edge
    tiles; a one-hot selection matrix S (built in one DVE op from
    precomputed per-edge dst-in-block offsets and edge weights) turns
    segment-sum into PE matmuls accumulated in PSUM per 128-row dst block.
  - Restart mix: out = agg + 0.1*h0 (0.9 is folded into edge weights on
    the host); h0*0.1 stays SBUF-resident.
  - Edge padding slots use an out-of-bounds gather index (skipped via
    bounds_check) and dst offset -1 (zero row in S) so every core runs the
    same instruction stream (SPMD) on a uniform cross-core tile schedule.
  - Final hop computes log_softmax per 128-node block and writes f32.
"""

import numpy as np
import ml_dtypes

BF16NP = ml_dtypes.bfloat16

# ---- problem constants (fixed; kernel.py must be self-contained) ----
N = 100000
E = 1600000
FEAT = 512
HID = 256
CLS = 128
KHOPS = 5
ALPHA = 0.1
NCORES = 8
NSH = N // NCORES                # 12500 nodes per core
NBLK = (NSH + 127) // 128        # 98 dst blocks per core
NSH_PAD = NBLK * 128             # 12544
SENT = N                         # out-of-bounds gather sentinel
CH = 32                          # edge tiles per gather chunk


def _mlp_node_blocks(nsh_pad):
    blocks = []
    n = 0
    while n < nsh_pad:
        nb = min(512, nsh_pad - n)
        blocks.append((n, nb))
        n += nb
    return blocks


def prep_edges(src, dst, w, n=N, ncores=NCORES, nsh=NSH, nblk=None):
    """Host-side edge preprocessing.

    Returns (eidx [ncores,128,NT] i32, edm [ncores,128,2*NT] f32
    (dst-in-block offsets | folded 0.9*weights), tile_block [NT] int, NT).
    Uniform cross-core schedule: per dst block b, tiles_b = max over cores of
    ceil(edges/128); each core pads its block edge list to tiles_b*128 slots.
    """
    if nblk is None:
        nblk = (nsh + 127) // 128
    src = np.asarray(src, np.int64)
    dst = np.asarray(dst, np.int64)
    w = np.asarray(w, np.float32)

    percore = []
    counts = np.zeros((ncores, nblk), np.int64)
    for c in range(ncores):
        lo, hi = c * nsh, (c + 1) * nsh
        m = (dst >= lo) & (dst < hi)
        s = src[m]
        d = dst[m] - lo
        ww = w[m] * (1.0 - ALPHA)
        o = np.argsort(d, kind="stable")
        s, d, ww = s[o], d[o], ww[o]
        counts[c] = np.bincount(d >> 7, minlength=nblk)
        percore.append((s, d, ww))

    tiles_b = np.maximum(np.ceil(counts / 128.0).astype(np.int64).max(axis=0), 1)
    nt = int(tiles_b.sum())
    tile_block = np.repeat(np.arange(nblk), tiles_b)
    blk_slot0 = np.zeros(nblk, np.int64)
    blk_slot0[1:] = np.cumsum(tiles_b)[:-1] * 128
    blk_slot0_full = np.zeros(nblk, np.int64)
    blk_slot0_full[1:] = np.cumsum(tiles_b)[:-1]

    eidx = np.zeros((ncores, nt * 128), np.int32)  # pad slots gather row 0 (S row is 0)
    edst = np.full((ncores, nt * 128), -1.0, np.float32)
    ewt = np.zeros((ncores, nt * 128), np.float32)
    for c in range(ncores):
        s, d, ww = percore[c]
        cnt = counts[c]
        # position of edge within its block
        off = np.arange(len(d), dtype=np.int64) - np.repeat(
            np.concatenate([[0], np.cumsum(cnt)[:-1]]), cnt
        )
        pos = np.repeat(blk_slot0[: len(cnt)], cnt) + off
        eidx[c, pos] = s.astype(np.int32)
        edst[c, pos] = (d & 127).astype(np.float32)
        ewt[c, pos] = ww

    def wrap(a, dt):
        # slot j of tile t -> partition j, column t
        return np.ascontiguousarray(
            a.reshape(ncores, nt, 128).transpose(0, 2, 1)
        ).astype(dt)

    edm = np.concatenate([wrap(edst, BF16NP), wrap(ewt, BF16NP)], axis=2)
    return wrap(eidx, np.int32), edm, tile_block, nt


def _hoist_excess_waits(nc, mybir, max_waits=1):
    """Walrus limits per-instruction sync-wait encoding slots (varies by
    instruction struct / AP rank). Hoist excess waits onto standalone
    EventSemaphore instructions inserted just before, on the same engine
    queue — semantically identical for in-order engine streams."""
    ctr = 0
    for bb in nc.main_func.blocks:
        out = []
        for ins in bb.instructions:
            si = ins.sync_info
            waits = list(si.on_wait) if (si and si.on_wait) else []
            if (not isinstance(ins, mybir.InstEventSemaphore)
                    and len(waits) > max_waits):
                hoist = waits[:len(waits) - max_waits]
                for w in hoist:
                    ev = mybir.InstEventSemaphore(
                        name=f"evhoist-{ctr}", engine=ins.engine,
                        ins=[], outs=[],
                        sync_info=mybir.SyncInfo(on_wait=[w], on_update=[]))
                    nc.inst_map[ev.name] = ev
                    out.append(ev)
                    ctr += 1
                si.on_wait = waits[len(waits) - max_waits:]
            out.append(ins)
        bb.instructions = out
    return ctr


def build_program(nt, tile_block, n=N, nsh=NSH, nblk=None, ncores=NCORES,
                  khops=KHOPS, feat=FEAT, hid=HID, cls=CLS, ch=CH,
                  table_f32=False):
    """Build the SPMD bass program (same instruction stream for all cores)."""
    import concourse.bass as bass
    import concourse.bacc as bacc
    import concourse.mybir as mybir
    import concourse.tile as tile
    from concourse.masks import make_identity

    if nblk is None:
        nblk = (nsh + 127) // 128
    nsh_pad = nblk * 128
    f32 = mybir.dt.float32
    bf = mybir.dt.bfloat16
    i32 = mybir.dt.int32
    AT = mybir.ActivationFunctionType
    OP = mybir.AluOpType
    kf = feat // 128   # k-chunks of layer 1
    fh = hid // 128    # feature chunks of hidden layer
    rg = [list(range(ncores))]

    nc = bacc.Bacc(trn_type="TRN2", num_devices=ncores,
                   dynamic_dma_scratch_size=49152)

    xT = nc.declare_dram_parameter("xT", [feat, nsh_pad], bf, isOutput=False)
    W1p = nc.declare_dram_parameter("W1", [feat, hid], bf, isOutput=False)
    W2p = nc.declare_dram_parameter("W2", [hid, cls], bf, isOutput=False)
    b1p = nc.declare_dram_parameter("b1", [128, fh], f32, isOutput=False)
    b2p = nc.declare_dram_parameter("b2", [128, 1], f32, isOutput=False)
    eidxp = nc.declare_dram_parameter("eidx", [128, nt], i32, isOutput=False)
    edmp = nc.declare_dram_parameter("edm", [128, 2 * nt], bf, isOutput=False)
    outp = nc.declare_dram_parameter("out", [nsh, cls], f32, isOutput=True)

    tdt = f32 if table_f32 else bf
    table = nc.dram_tensor("table", [n, cls], tdt)
    agin = nc.dram_tensor("agin", [nsh, cls], tdt)

    n_chunks = (nt + ch - 1) // ch
    # first/last tile flags per block
    first_of_blk = np.zeros(nt, bool)
    last_of_blk = np.zeros(nt, bool)
    prev = -1
    for t in range(nt):
        if tile_block[t] != prev:
            first_of_blk[t] = True
            if t > 0:
                last_of_blk[t - 1] = True
            prev = tile_block[t]
    last_of_blk[nt - 1] = True

    with tile.TileContext(nc) as tc:
        with (
            tc.tile_pool(name="persist", bufs=1) as pers,
            tc.tile_pool(name="mlpx", bufs=3) as mlpx,
            tc.tile_pool(name="mlph", bufs=3) as mlph,
            tc.tile_pool(name="mlpo", bufs=4) as mlpo,
            tc.tile_pool(name="psmm", bufs=2, space="PSUM") as psmm,
            tc.tile_pool(name="pstr", bufs=2, space="PSUM") as pstr,
            tc.tile_pool(name="gat", bufs=2) as gat,
            tc.tile_pool(name="sel", bufs=2) as sel,
            tc.tile_pool(name="acc", bufs=4, space="PSUM") as accp,
            tc.tile_pool(name="mix", bufs=4) as mixp,
            tc.tile_pool(name="sfx", bufs=4) as sfx,
            tc.tile_pool(name="red", bufs=6) as red,
        ):
            # ---- persistent setup ----
            # iota replicated per edge tile of a chunk: col (t, j) = j
            iota_i = pers.tile([128, ch * 128], i32)
            nc.gpsimd.iota(iota_i[:], pattern=[[0, ch], [1, 128]], base=0,
                           channel_multiplier=0)
            iota_bf = pers.tile([128, ch * 128], bf)
            nc.vector.tensor_copy(iota_bf[:], iota_i[:])
            ident = pers.tile([128, 128], bf)
            make_identity(nc, ident[:])

            w1s = pers.tile([128, kf * hid], bf)
            for k in range(kf):
                nc.sync.dma_start(out=w1s[:, k * hid:(k + 1) * hid],
                                  in_=W1p[k * 128:(k + 1) * 128, :])
            w2s = pers.tile([128, fh * cls], bf)
            for k in range(fh):
                nc.sync.dma_start(out=w2s[:, k * cls:(k + 1) * cls],
                                  in_=W2p[k * 128:(k + 1) * 128, :])
            b1s = pers.tile([128, fh], f32)
            nc.sync.dma_start(out=b1s[:], in_=b1p[:, :])
            b2s = pers.tile([128, 1], f32)
            nc.sync.dma_start(out=b2s[:], in_=b2p[:, :])

            eidx_s = pers.tile([128, nt], i32)
            nc.sync.dma_start(out=eidx_s[:], in_=eidxp[:, :])
            # dst offsets (cols 0:nt) and weights (cols nt:2nt) share one
            # tile/DMA so dependent instructions need a single wait.
            edm_s = pers.tile([128, 2 * nt], bf)
            nc.sync.dma_start(out=edm_s[:], in_=edmp[:, :])

            h0s = pers.tile([128, nsh_pad], bf)  # 0.1 * h0, block-major

            # ---- MLP over node blocks ----
            for (n0, nb) in _mlp_node_blocks(nsh_pad):
                xts = mlpx.tile([128, kf, 512], bf, tag="xts")
                for k in range(kf):
                    nc.sync.dma_start(out=xts[:, k, :nb],
                                      in_=xT[k * 128:(k + 1) * 128, n0:n0 + nb])
                h1s = mlph.tile([128, fh, 512], bf, tag="h1s")
                for fc in range(fh):
                    ph1 = psmm.tile([128, 512], f32, tag="ph")
                    for k in range(kf):
                        nc.tensor.matmul(
                            ph1[:, :nb],
                            lhsT=w1s[:, k * hid + fc * 128: k * hid + (fc + 1) * 128],
                            rhs=xts[:, k, :nb],
                            start=(k == 0), stop=(k == kf - 1))
                    nc.vector.tensor_scalar(
                        out=h1s[:, fc, :nb], in0=ph1[:, :nb],
                        scalar1=b1s[:, fc:fc + 1], scalar2=0.0,
                        op0=OP.add, op1=OP.max)
                ph2 = psmm.tile([128, 512], f32, tag="ph")
                for fc in range(fh):
                    nc.tensor.matmul(ph2[:, :nb],
                                     lhsT=w2s[:, fc * cls:(fc + 1) * cls],
                                     rhs=h1s[:, fc, :nb],
                                     start=(fc == 0), stop=(fc == fh - 1))
                h2t = mlpo.tile([128, 512], bf, tag="h2t")
                nc.vector.tensor_scalar(out=h2t[:, :nb], in0=ph2[:, :nb],
                                        scalar1=b2s[:, 0:1], scalar2=None, op0=OP.add)
                for sub in range(nb // 128):
                    b = (n0 + sub * 128) // 128
                    pt = pstr.tile([128, 128], bf, tag="pt")
                    nc.tensor.transpose(pt[:], h2t[:, sub * 128:(sub + 1) * 128],
                                        ident[:])
                    rows = min(128, nsh - b * 128)
                    if rows > 0:
                        nm = mlpo.tile([128, 128], tdt, tag="nm")
                        nc.vector.tensor_copy(nm[:], pt[:])
                        nc.sync.dma_start(out=agin[b * 128:b * 128 + rows, :],
                                          in_=nm[:rows, :])
                    nc.vector.tensor_scalar(
                        out=h0s[:, b * 128:(b + 1) * 128], in0=pt[:],
                        scalar1=ALPHA, scalar2=None, op0=OP.mult)

            # initial AllGather: table <- concat(agin)
            nc.gpsimd.collective_compute(
                "AllGather", mybir.AluOpType.bypass, replica_groups=rg,
                ins=[agin.ap().opt()], outs=[table.ap().opt()])

            # ---- propagation hops ----
            for hop in range(khops):
                last_hop = hop == khops - 1
                pcur = None
                for c in range(n_chunks):
                    t0 = c * ch
                    th = min(ch, nt - t0)
                    # HW indirect DMA honors ONE offset per partition per
                    # call, so gather one 128-edge tile per call.
                    gtb = gat.tile([128, ch * 128], tdt, tag="gt")
                    for j in range(th):
                        nc.gpsimd.indirect_dma_start(
                            out=gtb[:, j * 128:(j + 1) * 128],
                            out_offset=None,
                            in_=table[:, :],
                            in_offset=bass.IndirectOffsetOnAxis(
                                ap=eidx_s[:, t0 + j:t0 + j + 1], axis=0))
                    # one-hot selection matrices for the whole chunk in two
                    # DVE ops: S[(t,j), e] = w[e,t] * (j == dstl[e,t])
                    S = sel.tile([128, ch * 128], bf, tag="S")
                    S3 = S[:, :th * 128].rearrange("p (t j) -> p t j", j=128)
                    nc.vector.tensor_tensor(
                        out=S3, in0=iota_bf[:, :th * 128].rearrange(
                            "p (t j) -> p t j", j=128),
                        in1=edm_s[:, t0:t0 + th].to_broadcast([128, th, 128]),
                        op=OP.is_equal)
                    nc.vector.tensor_tensor(
                        out=S3, in0=S3,
                        in1=edm_s[:, nt + t0:nt + t0 + th].to_broadcast(
                            [128, th, 128]),
                        op=OP.mult)
                    for j in range(th):
                        t = t0 + j
                        b = int(tile_block[t])
                        if first_of_blk[t]:
                            pcur = accp.tile([128, 128], f32, tag="acc")
                        nc.tensor.matmul(pcur[:], lhsT=S[:, j * 128:(j + 1) * 128],
                                         rhs=gtb[:, j * 128:(j + 1) * 128],
                                         start=first_of_blk[t],
                                         stop=last_of_blk[t])
                        if last_of_blk[t]:
                            rows = min(128, nsh - b * 128)
                            if not last_hop:
                                mo = mixp.tile([128, 128], tdt, tag="mo")
                                nc.vector.tensor_tensor(
                                    out=mo[:], in0=pcur[:],
                                    in1=h0s[:, b * 128:(b + 1) * 128],
                                    op=OP.add)
                                nc.sync.dma_start(
                                    out=agin[b * 128:b * 128 + rows, :],
                                    in_=mo[:rows, :])
                            else:
                                fo = sfx.tile([128, 128], f32, tag="fo")
                                nc.vector.tensor_tensor(
                                    out=fo[:], in0=pcur[:],
                                    in1=h0s[:, b * 128:(b + 1) * 128],
                                    op=OP.add)
                                mx = red.tile([128, 1], f32, tag="mx")
                                nc.vector.reduce_max(mx[:], fo[:],
                                                     axis=mybir.AxisListType.X)
                                sh = sfx.tile([128, 128], f32, tag="sh")
                                nc.vector.tensor_scalar(
                                    out=sh[:], in0=fo[:], scalar1=mx[:, 0:1],
                                    scalar2=None, op0=OP.subtract)
                                ex = sfx.tile([128, 128], f32, tag="ex")
                                nc.scalar.activation(ex[:], sh[:], AT.Exp)
                                sm = red.tile([128, 1], f32, tag="sm")
                                nc.vector.reduce_sum(sm[:], ex[:],
                                                     axis=mybir.AxisListType.X)
                                rcp = red.tile([128, 1], f32, tag="rcp")
                                nc.vector.reciprocal(rcp[:], sm[:])
                                lg = red.tile([128, 1], f32, tag="lg")
                                nc.scalar.activation(lg[:], rcp[:], AT.Ln)
                                res = sfx.tile([128, 128], f32, tag="res")
                                nc.scalar.activation(res[:], sh[:], AT.Identity,
                                                     bias=lg[:, 0:1])
                                nc.sync.dma_start(
                                    out=outp[b * 128:b * 128 + rows, :],
                                    in_=res[:rows, :])
                if not last_hop:
                    nc.gpsimd.collective_compute(
                        "AllGather", mybir.AluOpType.bypass, replica_groups=rg,
                        ins=[agin.ap().opt()], outs=[table.ap().opt()])

    nc.finalize()
    return nc


def prep_inputs(x, edge_index, edge_weight, W1, b1, W2, b2):
    """Host preprocessing -> (in_maps, nt, tile_block)."""
    x = np.asarray(x, np.float32)
    ei = np.asarray(edge_index)
    ew = np.asarray(edge_weight, np.float32)
    W1 = np.asarray(W1, np.float32)
    b1 = np.asarray(b1, np.float32)
    W2 = np.asarray(W2, np.float32)
    b2 = np.asarray(b2, np.float32)

    eidx, edm, tile_block, nt = prep_edges(ei[0], ei[1], ew)

    W1b = W1.astype(BF16NP)
    W2b = W2.astype(BF16NP)
    b1c = np.ascontiguousarray(b1.reshape(HID // 128, 128).T).astype(np.float32)
    b2c = np.ascontiguousarray(b2.reshape(1, 128).T).astype(np.float32)

    in_maps = []
    for c in range(NCORES):
        xs = x[c * NSH:(c + 1) * NSH]                       # [12500, 512]
        xpad = np.zeros((NSH_PAD, FEAT), np.float32)
        xpad[:NSH] = xs
        xTb = np.ascontiguousarray(xpad.T).astype(BF16NP)    # [512, 12544]
        in_maps.append({
            "xT": xTb,
            "W1": W1b, "W2": W2b, "b1": b1c, "b2": b2c,
            "eidx": eidx[c], "edm": edm[c],
        })
    return in_maps, nt, tile_block


def kernel(**inputs) -> np.ndarray:
    from concourse.bass_utils import run_bass_kernel_spmd

    in_maps, nt, tile_block = prep_inputs(
        inputs["x"], inputs["edge_index"], inputs["edge_weight"],
        inputs["W1"], inputs["b1"], inputs["W2"], inputs["b2"])
    nc = build_program(nt, tile_block)
    res = run_bass_kernel_spmd(nc, in_maps, core_ids=list(range(NCORES)))
    outs = [res.results[c]["out"] for c in range(NCORES)]
    return np.concatenate(outs, axis=0).astype(np.float32)


if __name__ == "__main__":
    import jax
    jax.config.update("jax_platforms", "cpu")
    import sys
    sys.path.insert(0, "/root/problem")
    import reference

    inputs = reference.setup_inputs()
    out = kernel(**{k: np.asarray(v) for k, v in inputs.items()})
    ref = np.asarray(reference.reference(**inputs))
    rel = np.linalg.norm(out - ref) / np.linalg.norm(ref)
    print("Relative error:", rel)
